# revision 51
# baseline (speedup 1.0000x reference)
"""Conformer layer on 8 Trainium2 NeuronCores.

Sharding: core c handles batch b=c//2, token half sc=c%2 (512 tokens).
 - FFN1/conv/FFN2/LN stages: token-parallel on the 512 local tokens.
 - Attention: ALL 8 heads for the local 512 queries (k/v over all 1024
   tokens) -> the attention output is complete locally and no
   ReduceScatter is needed. Only a 16KB pair AllGather of the 16-token
   edge slivers feeds the depthwise-conv halo.
Cross-core traffic: pair AllGather of the post-FFN1 hidden state, pair
AllGather of conv edge slivers, 8-way AllReduce of BatchNorm stats.

All activations are channels-first ([channel, token]); matmul operands
are bf16 (single-pass LDWEIGHTS + FWL), PSUM accumulation f32, residual
stream f32. The rel-pos term bd is a banded matmul bounced through DRAM
with a row-stride mismatch realizing the rel-shift; per-core the
positional table is pre-shifted by 512*sc so the band offsets are
core-independent. Weights prefetch on the sync DMA queue a stage ahead.
"""

import numpy as np

import concourse.bass as bass
import concourse.mybir as mybir
import concourse.tile as tile
from concourse import bacc
from concourse.bass import ds, ts
from concourse.bass_utils import run_bass_kernel_spmd
from contextlib import ExitStack

F32 = mybir.dt.float32
F32R = mybir.dt.float32r
BF16 = mybir.dt.bfloat16
AF = mybir.ActivationFunctionType
ALU = mybir.AluOpType

D, DFF, H, DK, KCONV = 512, 2048, 8, 64, 31
B, T = 4, 1024
EPS = 1e-5
HT = 512            # tokens per core
WIN = 544           # conv window: 16 + 512 + 16
PB = 2047
BAND = 1152         # bd band width per q-chunk
BST = 1160          # bd dram row stride (elements)
NCORES = 8

PAIRS = [[0, 1], [2, 3], [4, 5], [6, 7]]
ALLG = [[0, 1, 2, 3, 4, 5, 6, 7]]


def _r(ap):
    return ap.bitcast(F32R)


def _emit(nc):
    def inp(name, shape, dt=F32):
        return nc.dram_tensor(name, list(shape), dt, kind="ExternalInput")

    x_d = inp("x_loc", (4, 128, HT), F32R)
    wf1_d = inp("wf1", (4, 128, DFF), BF16); bf1_d = inp("bf1", (128, 16))
    wf2_d = inp("wf2", (16, 128, D), BF16); bf2_d = inp("bf2", (128, 4))
    wq_d = inp("wq", (4, 128, D), BF16); bq_d = inp("bq", (128, 4))
    dqv_d = inp("dqv", (128, 4))
    wk_d = inp("wk", (4, 128, D), BF16); bk_d = inp("bk", (128, 4))
    wv_d = inp("wv", (4, 128, D), BF16); bv_d = inp("bvrow", (1, D))
    wp_d = inp("wp", (4, 128, D), BF16)
    wo_d = inp("wo", (8, 64, D), BF16); bo_d = inp("bo", (128, 4))
    posT_d = inp("posT", (4, 128, 2048), BF16)
    pw1_d = inp("pw1", (4, 128, 1024), BF16); bpw1_d = inp("bpw1", (128, 8))
    dwdg_d = inp("dwdiag", (4, 21, 128, 128), BF16)
    dwv_d = inp("dwv", (128, 4, KCONV))
    bng_d = inp("bng", (128, 4)); bnb_d = inp("bnb", (128, 4))
    pw2_d = inp("pw2", (4, 128, D), BF16); bpw2_d = inp("bpw2", (128, 4))
    cmask_d = inp("cmask", (1, WIN), BF16)
    wg1_d = inp("wg1", (4, 128, DFF), BF16); bg1_d = inp("bg1", (128, 16))
    wg2_d = inp("wg2", (16, 128, D), BF16); bg2_d = inp("bg2", (128, 4))
    g5_d = inp("g5", (128, 4)); b5_d = inp("b5", (128, 4))
    onc_d = inp("onc", (128, 1), F32R)
    idn_d = inp("idn", (128, 128), F32R)
    onv_d = inp("onv", (128, 64), BF16)

    out_d = nc.dram_tensor("out_loc", [4, 128, HT], F32, kind="ExternalOutput")

    cc_h_in = nc.dram_tensor("cc_h_in", [4, 128, HT], F32R)
    cc_h_out = nc.dram_tensor("cc_h_out", [8, 128, HT], F32R)
    cc_e_in = nc.dram_tensor("cc_e_in", [128, 4, 32], F32)
    cc_e_out = nc.dram_tensor("cc_e_out", [2, 128, 4, 32], F32)
    cc_bn_in = nc.dram_tensor("cc_bn_in", [128, 8], F32)
    cc_bn_out = nc.dram_tensor("cc_bn_out", [128, 8], F32)
    bd_d = [nc.dram_tensor(f"bd_{i}", [128 * BST], BF16) for i in range(32)]

    ln_uid = [0]

    with tile.TileContext(nc) as tc, ExitStack() as ctx:
        const = ctx.enter_context(tc.tile_pool(name="const", bufs=1))
        ones_c = const.tile([128, 1], F32R)
        nc.sync.dma_start(out=ones_c[:], in_=onc_d[:])
        eps1 = const.tile([1, 1], F32); nc.vector.memset(eps1[:], EPS)
        epsP = const.tile([128, 1], F32); nc.vector.memset(epsP[:], EPS)
        ident = const.tile([128, 128], F32R)
        nc.sync.dma_start(out=ident[:], in_=idn_d[:])
        g5_sb = const.tile([128, 4], F32)
        nc.sync.dma_start(out=g5_sb[:], in_=g5_d[:])
        b5_sb = const.tile([128, 4], F32)
        nc.sync.dma_start(out=b5_sb[:], in_=b5_d[:])
        ones_row = const.tile([65, 64], F32R)
        nc.vector.memset(ones_row[64:65, :].bitcast(F32), 1.0)

        act = ctx.enter_context(tc.tile_pool(name="act", bufs=1))
        # h_full (post-FFN1 hidden), padded with 16 zero tokens on both ends
        hfe_sb = act.tile([128, 4, 16 + T + 16], F32R)
        nc.vector.memset(hfe_sb[:, :, 0:16].bitcast(F32), 0.0)
        nc.vector.memset(hfe_sb[:, :, 16 + T:].bitcast(F32), 0.0)

        # ---- conv weights except dwdiag (prefetched during attention) ----
        wCV = ctx.enter_context(tc.tile_pool(name="wCV", bufs=1))
        pw1_sb = wCV.tile([128, 4, 1024], BF16)
        bp1_sb = wCV.tile([128, 8], F32)
        pw2_sb = wCV.tile([128, 4, D], BF16)
        bp2_sb = wCV.tile([128, 4], F32)
        bng_sb = wCV.tile([128, 4], F32)
        bnb_sb = wCV.tile([128, 4], F32)
        bo_sb = wCV.tile([128, 4], F32)
        cm_sb = wCV.tile([1, WIN], BF16)
        dwv_sb = wCV.tile([128, 4, KCONV], F32)

        # ---- attention weights (prefetched during FFN1; freed before conv) --
        wATT_cm = tc.tile_pool(name="wATT", bufs=1)
        wATT = wATT_cm.__enter__()
        wq_sb = wATT.tile([128, 4, D], BF16)
        wk_sb = wATT.tile([128, 4, D], BF16)
        wv_sb = wATT.tile([128, 4, D], BF16)
        wp_sb = wATT.tile([128, 4, D], BF16)
        posT_sb = wATT.tile([128, 4, 2048], BF16)
        wo_sb = wATT.tile([64, 8, D], BF16)
        bq_sb = wATT.tile([128, 4], F32)
        bk_sb = wATT.tile([128, 4], F32)
        dqv_sb = wATT.tile([128, 4], F32)
        bv_sb = wATT.tile([1, D], F32)
        onv_sb = wATT.tile([128, 64], BF16)
        o8 = wATT.tile([64, 8, HT], BF16)
        h_sb = wATT.tile([128, 4, HT], F32R)   # local post-FFN1 hidden
        xnq = wATT.tile([128, 4, HT], BF16)    # LN2 of the local tokens

        # ---------- LN helper (channels-first, 4 chunks of 128 channels) ----
        def emit_ln(x4, out4, W, blocks, sbp, psum_pool=None):
            ln_uid[0] += 1
            with ExitStack() as lctx:
                if psum_pool is None:
                    lnps = lctx.enter_context(
                        tc.tile_pool(name=f"lnps{ln_uid[0]}", bufs=1,
                                     space="PSUM"))
                else:
                    lnps = psum_pool
                x2 = sbp.tile([128, 4, W], F32R, tag="ln_sq", bufs=1)
                for b0, bw in blocks:
                    nc.vector.tensor_mul(x2[:, :, b0:b0 + bw],
                                         x4[:, :, b0:b0 + bw],
                                         x4[:, :, b0:b0 + bw])
                    pss = lnps.tile([1, bw], F32, tag="lns")
                    psq = lnps.tile([1, bw], F32, tag="lnq")
                    for c in range(4):
                        nc.tensor.matmul(pss[:], _r(ones_c[:]),
                                         _r(x4[:, c, b0:b0 + bw]),
                                         start=(c == 0), stop=(c == 3))
                    for c in range(4):
                        nc.tensor.matmul(psq[:], _r(ones_c[:]),
                                         _r(x2[:, c, b0:b0 + bw]),
                                         start=(c == 0), stop=(c == 3))
                    mng = sbp.tile([1, bw], F32, tag="ln_m")
                    nc.scalar.activation(mng[:], pss[:], AF.Copy,
                                         scale=-1.0 / D)
                    e2 = sbp.tile([1, bw], F32, tag="ln_e2")
                    nc.scalar.activation(e2[:], psq[:], AF.Copy, scale=1.0 / D)
                    var = sbp.tile([1, bw], F32, tag="ln_var")
                    nc.vector.tensor_mul(var[:], mng[:], mng[:])
                    nc.vector.tensor_sub(var[:], e2[:], var[:])
                    sd = sbp.tile([1, bw], F32, tag="ln_sd")
                    nc.scalar.activation(sd[:], var[:], AF.Sqrt, bias=eps1[:])
                    rec = sbp.tile([1, bw], F32, tag="ln_rs")
                    scr = sbp.tile([1, bw], F32, tag="ln_scr")
                    nc.vector.reciprocal_approx_accurate(rec[:], sd[:], scr[:])
                    nmr = sbp.tile([1, bw], F32, tag="ln_nm")
                    nc.vector.tensor_mul(nmr[:], mng[:], rec[:])
                    rb_t = sbp.tile([128, bw], F32, tag="ln_rb")
                    nc.gpsimd.partition_broadcast(rb_t[:], rec[:])
                    nb_t = sbp.tile([128, bw], F32, tag="ln_nb")
                    nc.gpsimd.partition_broadcast(nb_t[:], nmr[:])
                    for c in range(4):
                        o = out4[:, c, b0:b0 + bw]
                        nc.vector.tensor_mul(o, x4[:, c, b0:b0 + bw], rb_t[:])
                        nc.vector.tensor_add(o, o, nb_t[:])

        # ---------- FFN helper ----------
        def emit_ffn(xn, xres, out, w1, b1, w2, b2, pref):
            with tc.tile_pool(name=pref + "t", bufs=3) as tp, \
                 tc.tile_pool(name=pref + "ps", bufs=2, space="PSUM") as psp, \
                 tc.tile_pool(name=pref + "ph", bufs=1, space="PSUM") as php:
                psh = php.tile([128, 4, HT], F32)
                for j in range(16):
                    psy = psp.tile([128, HT], F32, tag="psy")
                    if j == 0:
                        # token-halved so the first matmuls start as soon as
                        # the LN's first block has been applied
                        for hs in (slice(0, 256), slice(256, 512)):
                            for c in range(4):
                                nc.tensor.matmul(psy[:, hs],
                                                 w1[:, c, ts(j, 128)],
                                                 xn[:, c, hs],
                                                 start=(c == 0), stop=(c == 3),
                                                 skip_group_check=True)
                    else:
                        for c in range(4):
                            nc.tensor.matmul(psy[:], w1[:, c, ts(j, 128)],
                                             xn[:, c, :],
                                             start=(c == 0), stop=(c == 3))
                    y1 = tp.tile([128, HT], BF16, tag="y1")
                    nc.scalar.activation(y1[:], psy[:], AF.Silu,
                                         bias=b1[:, j:j + 1])
                    for f in range(4):
                        nc.tensor.matmul(psh[:, f, :],
                                         w2[:, j, ts(f, 128)], y1[:],
                                         start=(j == 0), stop=(j == 15))
                for c in range(4):
                    nc.vector.scalar_tensor_tensor(
                        out=out[:, c, :], in0=psh[:, c, :],
                        scalar=b2[:, c:c + 1], in1=xres[:, c, :],
                        op0=ALU.add, op1=ALU.add)

        # ================= Stage A: FFN1 on local tokens =================
        with tc.tile_pool(name="stA", bufs=1) as stA, \
             tc.tile_pool(name="stAt", bufs=2) as stAt:
            x_sb = stA.tile([128, 4, HT], F32R)
            for c in range(4):
                nc.sync.dma_start(out=x_sb[:, c, :], in_=x_d[c])
            w1_sb = stA.tile([128, 4, DFF], BF16)
            for c in range(4):
                nc.sync.dma_start(out=w1_sb[:, c, :], in_=wf1_d[c])
            b1_sb = stA.tile([128, 16], F32)
            nc.sync.dma_start(out=b1_sb[:], in_=bf1_d[:])
            w2_sb = stA.tile([128, 16, D], BF16)
            for j in range(16):
                nc.sync.dma_start(out=w2_sb[:, j, :], in_=wf2_d[j])
            b2_sb = stA.tile([128, 4], F32)
            nc.sync.dma_start(out=b2_sb[:], in_=bf2_d[:])

            # prefetch attention weights behind FFN1's on the sync queue
            for c in range(4):
                nc.sync.dma_start(out=wq_sb[:, c, :], in_=wq_d[c])
                nc.sync.dma_start(out=wk_sb[:, c, :], in_=wk_d[c])
                nc.sync.dma_start(out=wv_sb[:, c, :], in_=wv_d[c])
                nc.sync.dma_start(out=wp_sb[:, c, :], in_=wp_d[c])
                nc.sync.dma_start(out=posT_sb[:, c, :], in_=posT_d[c])
            for hh in range(8):
                nc.sync.dma_start(out=wo_sb[:, hh, :], in_=wo_d[hh])
            nc.sync.dma_start(out=bq_sb[:], in_=bq_d[:])
            nc.sync.dma_start(out=bk_sb[:], in_=bk_d[:])
            nc.sync.dma_start(out=dqv_sb[:], in_=dqv_d[:])
            nc.sync.dma_start(out=bv_sb[:], in_=bv_d[:])
            nc.sync.dma_start(out=bo_sb[:], in_=bo_d[:])
            nc.sync.dma_start(out=onv_sb[:], in_=onv_d[:])
            # conv weights (less dwdiag) behind those
            for c in range(4):
                nc.sync.dma_start(out=pw1_sb[:, c, :], in_=pw1_d[c])
                nc.sync.dma_start(out=pw2_sb[:, c, :], in_=pw2_d[c])
            nc.sync.dma_start(out=bp1_sb[:], in_=bpw1_d[:])
            nc.sync.dma_start(out=bp2_sb[:], in_=bpw2_d[:])
            nc.sync.dma_start(out=bng_sb[:], in_=bng_d[:])
            nc.sync.dma_start(out=bnb_sb[:], in_=bnb_d[:])
            nc.sync.dma_start(out=cm_sb[:], in_=cmask_d[:])
            nc.sync.dma_start(out=dwv_sb[:], in_=dwv_d[:])

            xn1 = stA.tile([128, 4, HT], BF16)
            emit_ln(x_sb[:], xn1[:], HT, [(0, 256), (256, 256)], stAt)
            emit_ffn(xn1[:], x_sb[:], h_sb[:], w1_sb, b1_sb, w2_sb, b2_sb,
                     "f1")
            for c in range(4):
                nc.gpsimd.dma_start(out=cc_h_in[c], in_=h_sb[:, c, :])

        nc.gpsimd.collective_compute(
            "AllGather", ALU.bypass, ins=[cc_h_in[:]], outs=[cc_h_out[:]],
            replica_groups=PAIRS)

        pid = nc.vector.partition_id()
        woff = (pid % 2) * 512          # window start in hfe coords
        pidx = (pid + 1) % 2            # pair partner row in cc_e_out

        # ============ attention (all 8 heads on local 512 queries) ============
        with tc.tile_pool(name="attp", bufs=1) as attp:
            q_sb = attp.tile([128, 4, HT], BF16)
            qv_sb = attp.tile([128, 4, HT], BF16)
            k_sb = attp.tile([128, 4, T], BF16)
            v65 = attp.tile([128, 8, 8, 65], BF16)
            nc.gpsimd.dma_start(out=v65[:, :, :, 64:65],
                                in_=onv_sb[:].rearrange("p (a b) -> p a b",
                                                        a=8))
            p_sb = attp.tile([128, 4, 2048], BF16)

            # ---- pre-gather work that fills the AllGather window: LN2 of
            # the LOCAL tokens (depends only on locally computed h) and the
            # q/qv projections
            with tc.tile_pool(name="stBt", bufs=2) as stBt, \
                 tc.tile_pool(name="qpp", bufs=2, space="PSUM") as qpp:
                emit_ln(h_sb[:], xnq[:], HT, [(0, 256), (256, 256)], stBt)
                for m in range(4):
                    psq = qpp.tile([128, 512], F32, tag="psq")
                    for c in range(4):
                        nc.tensor.matmul(
                            psq[:], wq_sb[:, c, ts(m, 128)],
                            xnq[:, c, :],
                            start=(c == 0), stop=(c == 3))
                    nc.vector.tensor_scalar_add(
                        q_sb[:, m, :], psq[:], bq_sb[:, m:m + 1])
                    nc.vector.tensor_scalar_add(qv_sb[:, m, :],
                                                q_sb[:, m, :],
                                                dqv_sb[:, m:m + 1])

            # ---- p-projection (also independent of the AllGather) ----
            with tc.tile_pool(name="ppp", bufs=2, space="PSUM") as ppp:
                for hc in range(4):
                    for g in range(2):
                        psp_t = ppp.tile([128, 2, 512], F32, tag="psp")
                        for c in range(4):
                            for pc in range(2):
                                nc.tensor.matmul(
                                    psp_t[:, pc, :],
                                    wp_sb[:, c, ts(hc, 128)],
                                    posT_sb[:, c, ts(2 * g + pc, 512)],
                                    start=(c == 0), stop=(c == 3))
                        for pc in range(2):
                            if pc == 0:
                                nc.scalar.activation(
                                    p_sb[:, hc, ts(2 * g + pc, 512)],
                                    psp_t[:, pc, :], AF.Copy)
                            else:
                                nc.vector.tensor_copy(
                                    p_sb[:, hc, ts(2 * g + pc, 512)],
                                    psp_t[:, pc, :])

            with tc.tile_pool(name="bds", bufs=2) as bdp, \
                 tc.tile_pool(name="psb", bufs=1, space="PSUM") as psb:

                def emit_bd(h):
                    hc, hr = h // 2, 64 * (h % 2)
                    for qc in range(4):
                        base = 895 - 128 * qc
                        bdw = bdp.tile([128, BAND], BF16, tag="bdw")
                        for pi in range(3):
                            psB = psb.tile([128, 384], F32, tag="psB")
                            nc.tensor.matmul(
                                psB[:],
                                qv_sb[hr:hr + 64, hc, ts(qc, 128)],
                                p_sb[hr:hr + 64, hc,
                                     base + pi * 384: base + (pi + 1) * 384],
                                start=True, stop=True)
                            if pi < 2:
                                nc.vector.tensor_copy(
                                    bdw[:, pi * 384:(pi + 1) * 384], psB[:])
                            else:
                                nc.scalar.activation(
                                    bdw[:, pi * 384:(pi + 1) * 384], psB[:],
                                    AF.Copy)
                        nc.gpsimd.dma_start(
                            out=bd_d[h * 4 + qc][:].rearrange(
                                "(p w) -> p w", p=128)[:, 0:BAND],
                            in_=bdw[:])

                # all 8 heads' bd bands — fills the AllGather window
                for h in range(8):
                    emit_bd(h)

                # ---- gather h_full, LN2 over all tokens, k/v projections --
                for half in range(2):
                    for c in range(4):
                        nc.sync.dma_start(
                            out=hfe_sb[:, c,
                                       16 + half * HT:16 + (half + 1) * HT],
                            in_=cc_h_out[half * 4 + c])
                hf_sb = hfe_sb[:, :, 16:16 + T]
                with tc.tile_pool(name="stC", bufs=1) as stC, \
                     tc.tile_pool(name="stCt", bufs=2) as stCt:
                    xn2 = stC.tile([128, 4, T], BF16)
                    emit_ln(hf_sb, xn2[:], T, [(0, 512), (512, 512)], stCt)
                    with tc.tile_pool(name="qkp", bufs=2, space="PSUM") as kp_:
                        bvb_sb = stC.tile([128, D], F32)
                        nc.gpsimd.partition_broadcast(bvb_sb[:], bv_sb[:])
                        for m in range(4):
                            for th in range(2):
                                sl = slice(th * 512, (th + 1) * 512)
                                psk = kp_.tile([128, 512], F32, tag="psk")
                                for c in range(4):
                                    nc.tensor.matmul(
                                        psk[:], wk_sb[:, c, ts(m, 128)],
                                        xn2[:, c, sl],
                                        start=(c == 0), stop=(c == 3))
                                nc.vector.tensor_scalar_add(
                                    k_sb[:, m, sl], psk[:], bk_sb[:, m:m + 1])
                        for tq in range(8):
                            psv = kp_.tile([128, 512], F32, tag="psv")
                            for c in range(4):
                                nc.tensor.matmul(
                                    psv[:], xn2[:, c, ts(tq, 128)],
                                    wv_sb[:, c, :],
                                    start=(c == 0), stop=(c == 3))
                            nc.vector.tensor_add(
                                v65[:, tq, :, 0:64],
                                psv[:].rearrange("p (h d) -> p h d", h=8),
                                bvb_sb[:].rearrange("p (h d) -> p h d", h=8))

                # ---- attention heads ----
                with tc.tile_pool(name="bdsh", bufs=1) as shp, \
                     tc.tile_pool(name="atp", bufs=2) as atp, \
                     tc.tile_pool(name="atn", bufs=1) as atn, \
                     tc.tile_pool(name="pss", bufs=2, space="PSUM") as pss, \
                     tc.tile_pool(name="psav", bufs=2, space="PSUM") as psav, \
                     tc.tile_pool(name="psrb", bufs=1, space="PSUM") as psrb:

                    def emit_scores(h, bdsh):
                        hc, hr = h // 2, 64 * (h % 2)
                        psA = psav.tile([65, HT], F32, tag="psAV")
                        for kp in range(4):       # pairs of k-chunks
                            psS = pss.tile([128, 2, 512], F32, tag="psS")
                            for i in range(2):
                                kc = 2 * kp + i
                                nc.tensor.matmul(
                                    psS[:, i, :],
                                    k_sb[hr:hr + 64, hc, ts(kc, 128)],
                                    q_sb[hr:hr + 64, hc, :],
                                    start=True, stop=False,
                                    skip_group_check=True)
                                for qc in range(4):
                                    nc.tensor.matmul(
                                        _r(psS[:, i, ts(qc, 128)]),
                                        _r(bdsh[:, qc, ts(kc, 128)]),
                                        _r(ident[:]),
                                        is_transpose=True, start=False,
                                        stop=(qc == 3), skip_group_check=True)
                            probs = atp.tile([128, 2, 512], BF16, tag="probs")
                            nc.scalar.activation(
                                probs[:].rearrange("p a b -> p (a b)"),
                                psS[:].rearrange("p a b -> p (a b)"), AF.Exp)
                            for i in range(2):
                                nc.tensor.matmul(
                                    psA[:], v65[:, 2 * kp + i, h, :],
                                    probs[:, i, :],
                                    start=(kp == 0 and i == 0),
                                    stop=(kp == 3 and i == 1),
                                    skip_group_check=True)
                        s65 = atn.tile([65, HT], F32, tag="s65")
                        nc.scalar.activation(s65[64:65, :], psA[64:65, :],
                                             AF.Copy)
                        rc65 = atn.tile([65, HT], F32R, tag="rc65")
                        with nc.allow_low_precision(
                                reason="f32r recip feeds f32r bcast matmul"):
                            nc.vector.reciprocal(rc65[64:65, :], s65[64:65, :])
                        return psA, rc65

                    def finish_scores(h, psA, rc65):
                        rb_ps = psrb.tile([64, HT], F32, tag="rb")
                        nc.tensor.matmul(rb_ps[:], ones_row[64:65, :],
                                         rc65[64:65, :], start=True, stop=True)
                        rb_sb = atn.tile([64, HT], F32, tag="rb_sb")
                        nc.vector.tensor_copy(rb_sb[:], rb_ps[:])
                        nc.vector.tensor_mul(o8[:, h, :], psA[0:64, :],
                                             rb_sb[:])

                    pend = None
                    for h in range(8):
                        bdsh = shp.tile([128, 4, T], F32R, tag=f"sh{h % 2}")
                        for qc in range(4):
                            src = bass.AP(tensor=bd_d[h * 4 + qc], offset=128,
                                          ap=[[BST - 1, 128], [1, T]])
                            nc.gpsimd.dma_start(out=bdsh[:, qc, :], in_=src)
                        psA, rc65 = emit_scores(h, bdsh)
                        if pend is not None:
                            finish_scores(*pend)
                        pend = (h, psA, rc65)
                    finish_scores(*pend)

        # ============ out-projection + edge sliver exchange + conv ============
        late = ctx.enter_context(tc.tile_pool(name="late", bufs=1,
                                              side="right"))
        h2w = late.tile([128, 4, WIN], F32R)     # conv window hidden
        h3_sb = late.tile([128, 4, HT], F32R)    # post-conv hidden
        h4_sb = late.tile([128, 4, HT], F32R)    # post-FFN2 hidden
        es2 = late.tile([128, 2, 4, 32], F32)    # gathered edge slivers

        with tc.tile_pool(name="pso", bufs=2, space="PSUM") as psop, \
             tc.tile_pool(name="aot", bufs=1) as aot:
            es = aot.tile([128, 4, 32], F32)
            for f in range(4):
                pso = psop.tile([128, HT], F32, tag="pso")
                for hh in range(8):
                    nc.tensor.matmul(
                        pso[:], wo_sb[:, hh, ts(f, 128)], o8[:, hh, :],
                        start=(hh == 0), stop=(hh == 7),
                        skip_group_check=True)
                nc.vector.tensor_copy(es[:, f, 0:16], pso[:, 0:16])
                nc.vector.tensor_copy(es[:, f, 16:32], pso[:, 496:512])
                # middle of the conv window: local attn out + residual
                nc.vector.scalar_tensor_tensor(
                    out=h2w[:, f, 16:16 + HT], in0=pso[:],
                    scalar=bo_sb[:, f:f + 1],
                    in1=hfe_sb[:, f, ds(woff + 16, HT)].bitcast(F32),
                    op0=ALU.add, op1=ALU.add)
            nc.gpsimd.dma_start(out=cc_e_in[:], in_=es[:])

        nc.gpsimd.collective_compute(
            "AllGather", ALU.bypass, ins=[cc_e_in[:]], outs=[cc_e_out[:]],
            replica_groups=PAIRS)

        wATT_cm.__exit__(None, None, None)

        # FFN2 weights + dwdiag: DMA behind the sliver-AllGather window
        wG = ctx.enter_context(tc.tile_pool(name="wG", bufs=1, side="right"))
        wg1_sb = wG.tile([128, 4, DFF], BF16)
        bg1_sb = wG.tile([128, 16], F32)
        wg2_sb = wG.tile([128, 16, D], BF16)
        bg2_sb = wG.tile([128, 4], F32)

        with tc.tile_pool(name="dwp", bufs=1) as dwp, \
             tc.tile_pool(name="stF", bufs=1) as stF, \
             tc.tile_pool(name="stFt", bufs=2) as stFt:
            # edge slivers first, on the gpsimd queue, so they are not
            # stuck behind 12MB of weight DMAs on the sync queue
            for r in range(2):
                nc.gpsimd.dma_start(out=es2[:, r, :, :], in_=cc_e_out[r])
            dg = dwp.tile([128, 4, 21, 128], BF16)
            for c in range(4):
                nc.sync.dma_start(
                    out=dg[:, c, :, :],
                    in_=dwdg_d[c].rearrange("j p w -> p j w"))
            for c in range(4):
                nc.sync.dma_start(out=wg1_sb[:, c, :], in_=wg1_d[c])
            for j in range(16):
                nc.sync.dma_start(out=wg2_sb[:, j, :], in_=wg2_d[j])
            nc.sync.dma_start(out=bg1_sb[:], in_=bg1_d[:])
            nc.sync.dma_start(out=bg2_sb[:], in_=bg2_d[:])

            esv = es2[:].rearrange("p r f w -> p (r f w)")
            for f in range(4):
                nc.vector.scalar_tensor_tensor(
                    out=h2w[:, f, 0:16],
                    in0=esv[:, ds(pidx * 128 + f * 32 + 16, 16)],
                    scalar=bo_sb[:, f:f + 1],
                    in1=hfe_sb[:, f, ds(woff, 16)].bitcast(F32),
                    op0=ALU.add, op1=ALU.add)
                nc.vector.scalar_tensor_tensor(
                    out=h2w[:, f, 528:544],
                    in0=esv[:, ds(pidx * 128 + f * 32, 16)],
                    scalar=bo_sb[:, f:f + 1],
                    in1=hfe_sb[:, f, ds(woff + 528, 16)].bitcast(F32),
                    op0=ALU.add, op1=ALU.add)

            xn3 = stF.tile([128, 4, WIN], BF16)
            # middles first (independent of the sliver exchange), edges after.
            # LN3's stat PSUM pool stays open through pw1 so the late edge
            # blocks don't alias cvp1's banks (which would serialize them).
            ln3ps = tc.tile_pool(name="ln3ps", bufs=1, space="PSUM")
            ln3p = ln3ps.__enter__()
            emit_ln(h2w[:], xn3[:], WIN,
                    [(16, 256), (272, 256), (0, 16), (528, 16)], stFt,
                    psum_pool=ln3p)
            cmb = stF.tile([128, WIN], BF16)
            nc.gpsimd.partition_broadcast(cmb[:], cm_sb[:])
            glu = stF.tile([128, 4, WIN], BF16)
            with tc.tile_pool(name="cvp1", bufs=1, space="PSUM") as cps:
                for m in range(4):
                    psa = cps.tile([128, 2, 512], F32, tag="psa")
                    psg = cps.tile([128, 2, 512], F32, tag="psg")
                    # window col w -> psa[w // 272, w % 272]
                    segs = [(16, 0, 16, 256), (272, 1, 0, 256),
                            (0, 0, 0, 16), (528, 1, 256, 16)]
                    for (w0, half, p0, bw) in segs:
                        sl = slice(w0, w0 + bw)
                        for c in range(4):
                            nc.tensor.matmul(psa[:, half, p0:p0 + bw],
                                             pw1_sb[:, c, ts(m, 128)],
                                             xn3[:, c, sl],
                                             start=(c == 0), stop=(c == 3),
                                             skip_group_check=True)
                        for c in range(4):
                            nc.tensor.matmul(psg[:, half, p0:p0 + bw],
                                             pw1_sb[:, c, 512 + m * 128:
                                                    512 + (m + 1) * 128],
                                             xn3[:, c, sl],
                                             start=(c == 0), stop=(c == 3),
                                             skip_group_check=True)
                        sg = stFt.tile([128, 2, 512], F32, tag="sg", bufs=1)
                        nc.scalar.activation(sg[:, half, p0:p0 + bw],
                                             psg[:, half, p0:p0 + bw],
                                             AF.Sigmoid,
                                             bias=bp1_sb[:, 4 + m:5 + m])
                        nc.vector.scalar_tensor_tensor(
                            out=glu[:, m, sl], in0=psa[:, half, p0:p0 + bw],
                            scalar=bp1_sb[:, m:m + 1],
                            in1=sg[:, half, p0:p0 + bw],
                            op0=ALU.add, op1=ALU.mult)
                    for e0 in (0, 528):
                        nc.vector.tensor_mul(glu[:, m, e0:e0 + 16],
                                             glu[:, m, e0:e0 + 16],
                                             cmb[:, e0:e0 + 16])
            ln3ps.__exit__(None, None, None)
            # depthwise conv: taps 0..20 as diagonal matmuls on PE, taps
            # 21..30 as per-channel scalar_tensor_tensor chains on DVE
            NPE = 21
            acc = stF.tile([128, 4, HT], F32)
            with tc.tile_pool(name="dgp", bufs=2, space="PSUM") as dgp:
                for c in range(4):
                    accv = stFt.tile([128, HT], BF16, tag="accv")
                    nc.vector.tensor_scalar_mul(
                        accv[:], glu[:, c, 1 + NPE:1 + NPE + HT],
                        dwv_sb[:, c, NPE:NPE + 1])
                    for j in range(NPE + 1, KCONV):
                        nc.vector.scalar_tensor_tensor(
                            out=accv[:], in0=glu[:, c, 1 + j:1 + j + HT],
                            scalar=dwv_sb[:, c, j:j + 1], in1=accv[:],
                            op0=ALU.mult, op1=ALU.add)
                    psC = dgp.tile([128, HT], F32, tag="psC")
                    for j in range(NPE):
                        nc.tensor.matmul(psC[:], dg[:, c, j, :],
                                         glu[:, c, 1 + j:1 + j + HT],
                                         start=(j == 0), stop=(j == NPE - 1))
                    nc.vector.tensor_add(acc[:, c, :], psC[:], accv[:])
            # BN stats + 8-way AllReduce (global stats — local-core stats
            # were measured at 2e-2 rel err, over the gate)
            bnpack = stF.tile([128, 8], F32)
            for c in range(4):
                bst_t = stFt.tile([128, 6], F32, tag="bst")
                nc.vector.bn_stats(bst_t[:], acc[:, c, :])
                mv = stFt.tile([128, 2], F32, tag="mv")
                nc.vector.bn_aggr(mv[:], bst_t[:])
                nc.vector.tensor_copy(bnpack[:, 2 * c:2 * c + 1], mv[:, 0:1])
                nc.vector.scalar_tensor_tensor(
                    out=bnpack[:, 2 * c + 1:2 * c + 2], in0=mv[:, 0:1],
                    scalar=mv[:, 0:1], in1=mv[:, 1:2],
                    op0=ALU.mult, op1=ALU.add)
            nc.gpsimd.dma_start(out=cc_bn_in[:], in_=bnpack[:])
            nc.gpsimd.collective_compute(
                "AllReduce", ALU.add, ins=[cc_bn_in[:]], outs=[cc_bn_out[:]],
                replica_groups=ALLG)
            bnar = stF.tile([128, 8], F32)
            nc.gpsimd.dma_start(out=bnar[:], in_=cc_bn_out[:])
            ysl = stF.tile([128, 4, HT], BF16)
            for c in range(4):
                mg = stFt.tile([128, 1], F32, tag="mg")
                nc.scalar.activation(mg[:], bnar[:, 2 * c:2 * c + 1], AF.Copy,
                                     scale=1.0 / NCORES)
                e2 = stFt.tile([128, 1], F32, tag="e2c")
                nc.scalar.activation(e2[:], bnar[:, 2 * c + 1:2 * c + 2],
                                     AF.Copy, scale=1.0 / NCORES)
                vg = stFt.tile([128, 1], F32, tag="vg")
                nc.vector.tensor_mul(vg[:], mg[:], mg[:])
                nc.vector.tensor_sub(vg[:], e2[:], vg[:])
                sdc = stFt.tile([128, 1], F32, tag="sdc")
                nc.scalar.activation(sdc[:], vg[:], AF.Sqrt, bias=epsP[:])
                rs = stFt.tile([128, 1], F32, tag="rsc")
                nc.vector.reciprocal(rs[:], sdc[:])
                s1 = stFt.tile([128, 1], F32, tag="s1c")
                nc.vector.tensor_mul(s1[:], rs[:], bng_sb[:, c:c + 1])
                s2 = stFt.tile([128, 1], F32, tag="s2c")
                nc.vector.tensor_mul(s2[:], mg[:], s1[:])
                nc.vector.tensor_sub(s2[:], bnb_sb[:, c:c + 1], s2[:])
                nc.scalar.activation(ysl[:, c, :], acc[:, c, :], AF.Silu,
                                     scale=s1[:], bias=s2[:])
            with tc.tile_pool(name="cvp2", bufs=2, space="PSUM") as cps2:
                for f in range(4):
                    psw = cps2.tile([128, HT], F32, tag="psw")
                    for c in range(4):
                        nc.tensor.matmul(psw[:], pw2_sb[:, c, ts(f, 128)],
                                         ysl[:, c, :],
                                         start=(c == 0), stop=(c == 3))
                    nc.vector.scalar_tensor_tensor(
                        out=h3_sb[:, f, :], in0=psw[:],
                        scalar=bp2_sb[:, f:f + 1],
                        in1=h2w[:, f, 16:16 + HT], op0=ALU.add, op1=ALU.add)

        # ================= Stage G: FFN2 =================
        with tc.tile_pool(name="stG", bufs=1) as stG, \
             tc.tile_pool(name="stGt", bufs=2) as stGt:
            xn4 = stG.tile([128, 4, HT], BF16)
            emit_ln(h3_sb[:], xn4[:], HT, [(0, 256), (256, 256)], stGt)
            emit_ffn(xn4[:], h3_sb[:], h4_sb[:], wg1_sb, bg1_sb, wg2_sb,
                     bg2_sb, "f2")

        # ================= Stage H: LN5 + output =================
        with tc.tile_pool(name="stH", bufs=1) as stH, \
             tc.tile_pool(name="stHt", bufs=2) as stHt:
            xn5 = stH.tile([128, 4, HT], F32)
            emit_ln(h4_sb[:], xn5[:], HT, [(0, 256), (256, 256)], stHt)
            for c in range(4):
                nc.vector.tensor_scalar(xn5[:, c, :], xn5[:, c, :],
                                        g5_sb[:, c:c + 1], b5_sb[:, c:c + 1],
                                        ALU.mult, ALU.add)
                nc.sync.dma_start(out=out_d[c], in_=xn5[:, c, :])
    return nc


_CACHE = {}


def build_nc():
    if "nc" not in _CACHE:
        nc = bacc.Bacc("TRN2", target_bir_lowering=False, debug=False,
                       num_devices=NCORES)
        _emit(nc)
        nc.compile()
        _CACHE["nc"] = nc
    return _CACHE["nc"]


def _chunk_cf(a2d):
    """[Dany, W] -> [Dany//128, 128, W] chunk-major channels-first."""
    d, w = a2d.shape
    return np.ascontiguousarray(a2d.reshape(d // 128, 128, w), dtype=np.float32)


def _bf(a):
    import ml_dtypes
    return np.asarray(a).astype(ml_dtypes.bfloat16)


def round_f32r(a):
    import ml_dtypes
    a = np.asarray(a, dtype=np.float32)
    hi = a.astype(ml_dtypes.bfloat16).astype(np.float32)
    lo = (a - hi).astype(ml_dtypes.bfloat16).astype(np.float32)
    return (hi + lo).astype(np.float32)


def _pcol(vec):
    """[Dout] per-channel vector -> [128, Dout//128] (partition, chunk)."""
    n = vec.shape[0]
    return np.ascontiguousarray(vec.reshape(n // 128, 128).T, dtype=np.float32)


def make_in_maps(inputs):
    inputs = {k: np.asarray(v, dtype=np.float32) for k, v in inputs.items()}
    x = inputs["x"]; pos_emb = inputs["pos_emb"]
    ln1_g, ln1_b = inputs["ln1_g"], inputs["ln1_b"]
    ln2_g, ln2_b = inputs["ln2_g"], inputs["ln2_b"]
    ln3_g, ln3_b = inputs["ln3_g"], inputs["ln3_b"]
    ln4_g, ln4_b = inputs["ln4_g"], inputs["ln4_b"]
    ln5_g, ln5_b = inputs["ln5_g"], inputs["ln5_b"]

    w1f = ln1_g[:, None] * inputs["ff1_w1"]
    b1f = inputs["ff1_b1"] + ln1_b @ inputs["ff1_w1"]
    w2f = 0.5 * inputs["ff1_w2"]; b2f = 0.5 * inputs["ff1_b2"]
    wg1f = ln4_g[:, None] * inputs["ff2_w1"]
    bg1f = inputs["ff2_b1"] + ln4_b @ inputs["ff2_w1"]
    wg2f = 0.5 * inputs["ff2_w2"]; bg2f = 0.5 * inputs["ff2_b2"]

    s = DK ** -0.5
    pos_u_f = inputs["pos_u"].reshape(D); pos_v_f = inputs["pos_v"].reshape(D)
    wqf = s * (ln2_g[:, None] * inputs["wq"])
    bqf = s * (inputs["bq"] + ln2_b @ inputs["wq"] + pos_u_f)
    dqvf = s * (pos_v_f - pos_u_f)
    wkf = ln2_g[:, None] * inputs["wk"]
    bkf = inputs["bk"] + ln2_b @ inputs["wk"]
    wvf = ln2_g[:, None] * inputs["wv"]
    bvf = inputs["bv"] + ln2_b @ inputs["wv"]

    pw1f = (inputs["pw1_w"] * ln3_g[None, :]).T            # [512, 1024]
    bpw1f = inputs["pw1_b"] + inputs["pw1_w"] @ ln3_b      # [1024]
    dwwf = inputs["dw_w"][:, 0, :]                         # [512, 31]
    # PE taps 0..20 as diagonal matrices; DVE taps 21..30 as scalars
    dwdiag = np.zeros((4, 21, 128, 128), dtype=np.float32)
    ar = np.arange(128)
    for c4 in range(4):
        for j4 in range(21):
            dwdiag[c4, j4, ar, ar] = dwwf[c4 * 128:(c4 + 1) * 128, j4]
    dwv = np.ascontiguousarray(
        dwwf.reshape(4, 128, KCONV).transpose(1, 0, 2), dtype=np.float32)
    pw2f = inputs["pw2_w"].T                               # [512, 512]

    base = {
        "wf1": _bf(_chunk_cf(w1f)), "bf1": _pcol(b1f),
        "wf2": _bf(_chunk_cf(w2f)), "bf2": _pcol(b2f),
        "wg1": _bf(_chunk_cf(wg1f)), "bg1": _pcol(bg1f),
        "wg2": _bf(_chunk_cf(wg2f)), "bg2": _pcol(bg2f),
        "wq": _bf(_chunk_cf(wqf)), "bq": _pcol(bqf),
        "dqv": _pcol(dqvf),
        "wk": _bf(_chunk_cf(wkf)), "bk": _pcol(bkf),
        "wv": _bf(_chunk_cf(wvf)),
        "bvrow": np.ascontiguousarray(bvf.reshape(1, D), dtype=np.float32),
        "wp": _bf(_chunk_cf(inputs["wp"])),
        "wo": _bf(np.ascontiguousarray(inputs["wo"].reshape(8, 64, D),
                                       dtype=np.float32)),
        "bo": _pcol(inputs["bo"]),
        "pw1": _bf(_chunk_cf(pw1f)), "bpw1": _pcol(bpw1f),
        "dwdiag": _bf(dwdiag), "dwv": dwv,
        "bng": _pcol(inputs["bn_g"]), "bnb": _pcol(inputs["bn_b"]),
        "pw2": _bf(_chunk_cf(pw2f)), "bpw2": _pcol(inputs["pw2_b"]),
        "g5": _pcol(ln5_g), "b5": _pcol(ln5_b),
        "onc": np.ones((128, 1), dtype=np.float32),
        "idn": np.eye(128, dtype=np.float32),
        "onv": _bf(np.ones((128, 64), dtype=np.float32)),
    }

    posT_full = pos_emb[0].T                               # [512, 2047]
    in_maps = []
    for c in range(NCORES):
        b, sc = c // 2, c % 2
        m = dict(base)
        xb = x[b, sc * HT:(sc + 1) * HT, :].T               # [512, 512]
        m["x_loc"] = round_f32r(_chunk_cf(xb))
        # per-core pre-shifted positional table: posT_core[:, j] =
        # posT_full[:, j - 512*sc]
        posT = np.zeros((D, 2048), dtype=np.float32)
        lo = 512 * sc
        posT[:, lo:lo + min(PB, 2048 - lo)] = posT_full[:, :min(PB, 2048 - lo)]
        m["posT"] = _bf(_chunk_cf(posT))
        cmask = np.ones((1, WIN), dtype=np.float32)
        if sc == 0:
            cmask[0, :16] = 0.0
        else:
            cmask[0, WIN - 16:] = 0.0
        m["cmask"] = _bf(cmask)
        in_maps.append(m)
    return in_maps


def assemble_out(results):
    out = np.empty((B, T, D), dtype=np.float32)
    for c in range(NCORES):
        b, sc = c // 2, c % 2
        ol = np.asarray(results[c]["out_loc"])              # [4, 128, 512]
        out[b, sc * HT:(sc + 1) * HT, :] = ol.reshape(D, HT).T
    return out


def kernel(**inputs):
    in_maps = make_in_maps(inputs)
    nc = build_nc()
    res = run_bass_kernel_spmd(nc, in_maps, list(range(NCORES)))
    return assemble_out(res.results)
